# revision 21
# baseline (speedup 1.0000x reference)
"""UR-LSTM forward kernel for Trainium2 (8 NeuronCores).

Strategy (sequence-parallel with warmup):
  The UR-LSTM state is contractive, so a chunk of the sequence can be
  computed to tolerance by starting W steps earlier from zero state.
  T=1024 is split into 16 chunks of 64 output steps; each of the 8 cores
  runs 2 independent chains of S = 64 + 12 steps (chunk 0's 12 warmup
  steps are zero-padded x, which keeps the state exactly zero).

  Per step, per chain (B=128 full batch on every core):
    gates[2048, 128]: 4 h-chunk matmuls per 128-gate tile (PE, bf16) plus
    the K=11 x/bias contribution packed as 4 concurrent 32-row tile_position
    matmuls (one per row-group).  Bias b and the UR-LSTM fb offsets are
    folded into the ones row.  f,r share a 2-bank PSUM tile, o and u get
    1-bank tiles (separate pools so write-after-read clears early).
    Elementwise is bf16 throughout (2x/4x DVE modes), split across ScalarE
    (activations), VectorE, and GpSimd.
    h_t is written into a [128, k(4), j(4), b(128)] ring tile; every 4
    steps the output projection y = W_out @ h + b_out runs as 4 N=512
    matmuls plus a rank-1 matmul for the bias.
"""

import numpy as np
import ml_dtypes

B, T, I, H = 128, 1024, 10, 512
G4 = 4 * H  # 2048
NCORES = 8
NCHUNK = 16
C_OUT = T // NCHUNK  # 64
W_WARM = 8
S_STEPS = C_OUT + W_WARM  # 76
NGRP = S_STEPS // 4  # 19
WGRP = W_WARM // 4  # 3 warmup groups
YGRP = NGRP - WGRP  # 16 output groups
KCH = 4  # h-chunks per gate tile (x handled via packed 32-row matmuls)
GT = 16  # gate tiles of 128
PAIR_FR = True  # pack f/r x-matmuls as cross-bank row-group pairs
PACKW = 64  # strip width for the x/bias weights (strips at 0 and 64)


def _xstrip(gt):
    # f tiles (0-3) strip 0, r tiles (4-7) strip 64 (their PSUM banks
    # differ, so the pair of closers can run in different row groups);
    # u/o tiles all strip 0 (serial closers)
    return 64 if 4 <= gt < 8 else 0

_cache = {}


def _build_nc():
    import concourse.bacc as bacc
    import concourse.mybir as mybir
    import concourse.tile as tile

    dt = mybir.dt
    f32, bf16 = dt.float32, dt.bfloat16
    AF = mybir.ActivationFunctionType
    OP = mybir.AluOpType
    S = S_STEPS

    nc = bacc.Bacc(None, target_bir_lowering=False)

    w_d = nc.dram_tensor("w", [128, KCH * GT * 128], bf16, kind="ExternalInput")
    wx_d = nc.dram_tensor("wx", [128, G4], bf16, kind="ExternalInput")
    wy_d = nc.dram_tensor("wy", [128, 4 * 10], bf16, kind="ExternalInput")
    wyb_d = nc.dram_tensor("wyb", [10, 1], f32, kind="ExternalInput")
    x_d = [
        nc.dram_tensor(f"x{c}", [128, S * 128], bf16, kind="ExternalInput")
        for c in range(2)
    ]
    y_d = [
        nc.dram_tensor(f"y{c}", [YGRP, 10, 512], f32, kind="ExternalOutput")
        for c in range(2)
    ]

    with tile.TileContext(nc) as tc:
        with (
            tc.tile_pool(name="const", bufs=1) as const,
            tc.tile_pool(name="hw", bufs=3) as hwp,
            tc.tile_pool(name="ew", bufs=2) as ew,
            tc.tile_pool(name="frp", bufs=2, space="PSUM") as frp,
            tc.tile_pool(name="obp", bufs=2, space="PSUM") as obp,
            tc.tile_pool(name="ubp", bufs=1, space="PSUM") as ubp,
            tc.tile_pool(name="ypsum", bufs=1, space="PSUM") as ypp,
            tc.tile_pool(name="yout", bufs=2) as youtp,
        ):
            wbuf = const.tile([128, KCH * GT * 128], bf16, tag="wbuf")
            nc.sync.dma_start(wbuf[:], w_d[:])
            wxbuf = const.tile([128, G4], bf16, tag="wxbuf")
            nc.sync.dma_start(wxbuf[:], wx_d[:])
            wybuf = const.tile([128, 4 * 10], bf16, tag="wybuf")
            nc.sync.dma_start(wybuf[:], wy_d[:])
            wyb = const.tile([10, 1], f32, tag="wyb")
            nc.sync.dma_start(wyb[:], wyb_d[:])

            xb = []
            cbuf = []
            hprev = []
            for c in range(2):
                t = const.tile([128, S * 128], bf16, tag=f"xb{c}")
                nc.sync.dma_start(t[:], x_d[c][:])
                xb.append(t)
                ct = const.tile([128, H], bf16, tag=f"cbuf{c}")
                nc.vector.memset(ct[:], 0.0)
                cbuf.append(ct)
                ht = hwp.tile([128, 4, 4, 128], bf16, tag=f"hw{c}")
                nc.vector.memset(ht[:], 0.0)
                hprev.append(ht)

            cur = [hprev[0], hprev[1]]
            st = [None, None]  # per-chain (so, hwin tile) between phases

            def xmm(c, s, out, gt, tp):
                base = _xstrip(gt)
                nc.tensor.matmul(
                    out,
                    lhsT=wxbuf[base : base + PACKW, gt * 128 : (gt + 1) * 128],
                    rhs=xb[c][base : base + PACKW, s * 128 : (s + 1) * 128],
                    start=False,
                    stop=True,
                    tile_position=(base, 0) if tp else None,
                )

            def hmms(c, s, prev, out, gt):
                jp = (s - 1) % 4
                for k in range(KCH):
                    nc.tensor.matmul(
                        out,
                        lhsT=wbuf[:, (k * GT + gt) * 128 : (k * GT + gt + 1) * 128],
                        rhs=prev[:, k, jp, :],
                        start=(k == 0),
                        stop=False,
                    )

            def mm_block(c, s, prev, bank, gt0, n):
                # per gate tile: 4 h-chunk matmuls, then its x/bias matmul
                # closing the accumulation group.  For the f/r bank the
                # closers of the pair (gt, gt+4) are adjacent: they write
                # different PSUM banks from different PE row groups, so
                # they run concurrently.
                if PAIR_FR and n == 8:
                    for pi in range(4):
                        ga, gb = gt0 + pi, gt0 + pi + 4
                        oa = bank[:, pi * 128 : (pi + 1) * 128]
                        ob_ = bank[:, (pi + 4) * 128 : (pi + 5) * 128]
                        hmms(c, s, prev, oa, ga)
                        hmms(c, s, prev, ob_, gb)
                        xmm(c, s, oa, ga, True)
                        xmm(c, s, ob_, gb, True)
                else:
                    for i in range(n):
                        gt = gt0 + i
                        out = bank[:, i * 128 : (i + 1) * 128]
                        hmms(c, s, prev, out, gt)
                        xmm(c, s, out, gt, False)

            def ph1(c, s):
                # u matmuls first (tanh-u unblocks the DVE chain early),
                # then f/r, then o; DVE chain last
                if s % 4 == 0:
                    cur[c] = hwp.tile(
                        [128, 4, 4, 128], bf16, tag=f"hw{c}", name=f"hwc{c}"
                    )
                prev = hprev[c]

                ub = ubp.tile([128, 512], f32, tag="ub")
                mm_block(c, s, prev, ub, 8, 4)
                tu = ew.tile([128, 512], bf16, tag="tu")
                nc.scalar.activation(tu[:], ub[:], AF.Tanh)

                fr = frp.tile([128, 1024], f32, tag="fr")
                mm_block(c, s, prev, fr, 0, 8)
                sfr = ew.tile([128, 1024], bf16, tag="sfr")
                nc.scalar.activation(sfr[:], fr[:], AF.Sigmoid)

                ob = obp.tile([128, 512], f32, tag="ob")
                mm_block(c, s, prev, ob, 12, 4)
                so = ew.tile([128, 512], bf16, tag="so")
                nc.scalar.activation(so[:], ob[:], AF.Sigmoid)

                # late elementwise chain, all on DVE (GpSimd shares its SBUF
                # port with VectorE — concurrent GpSimd work slows DVE ~2.7x):
                #   w = c - tanh(u); fw = f*w; pw = f*fw (= f^2*w)
                #   d = fw - pw (= f(1-f)w); e3 = 2r*d; c' = (tu + pw) + e3
                # equals c' = tu + g*(c - tu) with g = f^2 + 2rf(1-f)
                fg = sfr[:, 0:512]
                rgv = sfr[:, 512:1024]
                w_ = ew.tile([128, 512], bf16, tag="w_")
                fw = ew.tile([128, 512], bf16, tag="fw")
                rg2 = ew.tile([128, 512], bf16, tag="rg2")
                pw = ew.tile([128, 512], bf16, tag="pw")
                dd = ew.tile([128, 512], bf16, tag="dd")
                ca = ew.tile([128, 512], bf16, tag="ca")
                e3 = ew.tile([128, 512], bf16, tag="e3")
                nc.vector.tensor_tensor(w_[:], cbuf[c][:], tu[:], OP.subtract)
                nc.vector.tensor_tensor(fw[:], fg, w_[:], OP.mult)
                nc.vector.tensor_tensor(rg2[:], rgv, rgv, OP.add)
                nc.vector.tensor_tensor(pw[:], fg, fw[:], OP.mult)
                nc.vector.tensor_tensor(dd[:], fw[:], pw[:], OP.subtract)
                nc.vector.tensor_tensor(ca[:], tu[:], pw[:], OP.add)
                nc.vector.tensor_tensor(e3[:], rg2[:], dd[:], OP.mult)
                nc.vector.tensor_tensor(cbuf[c][:], ca[:], e3[:], OP.add)

                st[c] = (so, cur[c])
                hprev[c] = cur[c]

            def ph2h(c, s):
                # tanh(c) and h write for chain c's step s — emitted at the
                # head of the other chain's half so it clears the DVE FIFO
                # before that chain's own late chain queues up
                so, ht = st[c]
                tc2 = ew.tile([128, 512], bf16, tag="tc2")
                nc.scalar.activation(tc2[:], cbuf[c][:], AF.Tanh)
                nc.vector.tensor_tensor(ht[:, :, s % 4, :], so[:], tc2[:], OP.mult)

            def ph2y(c, s):
                # output projection for chain c's step s — PE hits these
                # right between the two chains' matmul phases
                j = s % 4
                g = s // 4
                if not (j == 3 and g >= WGRP):
                    return
                ht = st[c][1]
                yp = ypp.tile([10, 512], f32, tag="yp")
                for k in range(4):
                    nc.tensor.matmul(
                        yp[:],
                        lhsT=wybuf[:, k * 10 : (k + 1) * 10],
                        rhs=ht[:, k, :, :],
                        start=(k == 0),
                        stop=(k == 3),
                    )
                yo = youtp.tile([10, 512], f32, tag="yo")
                nc.vector.tensor_scalar_add(yo[:], yp[:], wyb[:])
                nc.sync.dma_start(y_d[c][g - WGRP], yo[:])

            # interleave: each chain's tanh(c)/h runs at the head of the
            # other chain's matmul phase; y-projection fills the gap
            # between the two phases
            for s in range(S):
                for c in (0, 1):
                    o = 1 - c
                    sprev = s - 1 if c == 0 else s
                    if sprev >= 0:
                        ph2h(o, sprev)
                    ph1(c, s)
                    if sprev >= 0:
                        ph2y(o, sprev)
            ph2h(1, S - 1)
            ph2y(1, S - 1)

    nc.compile()
    return nc


def _prep(inputs):
    x = np.asarray(inputs["x"], np.float32)
    W_ih = np.asarray(inputs["W_ih"], np.float32)
    W_hh = np.asarray(inputs["W_hh"], np.float32)
    b = np.asarray(inputs["b"], np.float32)
    fb = np.asarray(inputs["fb"], np.float32)
    W_out = np.asarray(inputs["W_out"], np.float32)
    b_out = np.asarray(inputs["b_out"], np.float32)
    bf = ml_dtypes.bfloat16

    bias_col = b.copy()
    bias_col[0:H] += fb
    bias_col[H : 2 * H] -= fb

    # h-recurrence weights: [512, 2048] -> per (k, gt) 128x128 lhsT tiles
    w_host = (
        W_hh.T.reshape(KCH, 128, GT, 128).transpose(1, 0, 2, 3).reshape(128, -1)
    ).astype(bf)

    # x/bias weights in row-group strips (r tiles in the 64-strip)
    wx = np.zeros((128, G4), np.float32)
    for gt in range(GT):
        base = 64 if 4 <= gt < 8 else 0
        wx[base : base + I, gt * 128 : (gt + 1) * 128] = W_ih.T[
            :, gt * 128 : (gt + 1) * 128
        ]
        wx[base + I, gt * 128 : (gt + 1) * 128] = bias_col[gt * 128 : (gt + 1) * 128]
    wx_host = wx.astype(bf)

    wy_host = (
        W_out.T.reshape(4, 128, 10).transpose(1, 0, 2).reshape(128, 40).astype(bf)
    )
    wyb_host = b_out.reshape(10, 1).astype(np.float32)

    xc = []
    for jc in range(NCHUNK):
        t0 = jc * C_OUT - W_WARM
        arr = np.zeros((128, S_STEPS * 128), np.float32)
        real0 = max(0, -t0)  # leading pad steps (chunk 0 only)
        xs = x[:, max(t0, 0) : jc * C_OUT + C_OUT, :]  # [128, S-real0, 10]
        a3 = arr.reshape(128, S_STEPS, 128)
        for base in (0, 64):
            a3[base : base + I, real0:] = xs.transpose(2, 1, 0)
            a3[base + I, real0:] = 1.0
        xc.append(arr.astype(bf))
    return w_host, wx_host, wy_host, wyb_host, xc


def _in_maps(inputs):
    w_host, wx_host, wy_host, wyb_host, xc = _prep(inputs)
    in_maps = []
    for core in range(NCORES):
        in_maps.append(
            {
                "w": w_host,
                "wx": wx_host,
                "wy": wy_host,
                "wyb": wyb_host,
                "x0": xc[2 * core],
                "x1": xc[2 * core + 1],
            }
        )
    return in_maps


def kernel(**inputs):
    from concourse.bass_utils import run_bass_kernel_spmd

    if "nc" not in _cache:
        _cache["nc"] = _build_nc()
    nc = _cache["nc"]

    in_maps = _in_maps(inputs)
    res = run_bass_kernel_spmd(nc, in_maps, list(range(NCORES))).results

    y = np.zeros((B, T, 10), np.float32)
    for jc in range(NCHUNK):
        core, chain = jc // 2, jc % 2
        yj = np.asarray(res[core][f"y{chain}"], np.float32)  # [16, 10, 512]
        yj = yj.reshape(YGRP, 10, 4, 128).transpose(3, 0, 2, 1).reshape(128, C_OUT, 10)
        y[:, jc * C_OUT : (jc + 1) * C_OUT, :] = yj
    return y


# revision 22
# speedup vs baseline: 1.3749x; 1.3749x over previous
"""UR-LSTM forward kernel for Trainium2 (8 NeuronCores).

Strategy (sequence-parallel with warmup):
  The UR-LSTM state is contractive, so a chunk of the sequence can be
  computed to tolerance by starting W steps earlier from zero state.
  T=1024 is split into 16 chunks of 64 output steps; each of the 8 cores
  runs 2 independent chains of S = 64 + 12 steps (chunk 0's 12 warmup
  steps are zero-padded x, which keeps the state exactly zero).

  Per step, per chain (B=128 full batch on every core):
    gates[2048, 128]: 4 h-chunk matmuls per 128-gate tile (PE, bf16) plus
    the K=11 x/bias contribution packed as 4 concurrent 32-row tile_position
    matmuls (one per row-group).  Bias b and the UR-LSTM fb offsets are
    folded into the ones row.  f,r share a 2-bank PSUM tile, o and u get
    1-bank tiles (separate pools so write-after-read clears early).
    Elementwise is bf16 throughout (2x/4x DVE modes), split across ScalarE
    (activations), VectorE, and GpSimd.
    h_t is written into a [128, k(4), j(4), b(128)] ring tile; every 4
    steps the output projection y = W_out @ h + b_out runs as 4 N=512
    matmuls plus a rank-1 matmul for the bias.
"""

import numpy as np
import ml_dtypes

B, T, I, H = 128, 1024, 10, 512
G4 = 4 * H  # 2048
NCORES = 8
NCHUNK = 16
C_OUT = T // NCHUNK  # 64
W_WARM = 8
S_STEPS = C_OUT + W_WARM  # 76
NGRP = S_STEPS // 4  # 19
WGRP = W_WARM // 4  # 3 warmup groups
YGRP = NGRP - WGRP  # 16 output groups
KCH = 4  # h-chunks per gate tile (x handled via packed 32-row matmuls)
GT = 16  # gate tiles of 128
PACKX = 1  # concurrent row-group tiles for the x/bias matmuls (1, 2, or 4)
PACKW = 128 // PACKX  # strip width

_cache = {}


def _build_nc():
    import concourse.bacc as bacc
    import concourse.mybir as mybir
    import concourse.tile as tile

    dt = mybir.dt
    f32, bf16 = dt.float32, dt.bfloat16
    AF = mybir.ActivationFunctionType
    OP = mybir.AluOpType
    S = S_STEPS

    nc = bacc.Bacc(None, target_bir_lowering=False)

    w_d = nc.dram_tensor("w", [128, KCH * GT * 128], bf16, kind="ExternalInput")
    wx_d = nc.dram_tensor("wx", [128, G4], bf16, kind="ExternalInput")
    wy_d = nc.dram_tensor("wy", [128, 4 * 10], bf16, kind="ExternalInput")
    wyb_d = nc.dram_tensor("wyb", [10, 1], f32, kind="ExternalInput")
    x_d = [
        nc.dram_tensor(f"x{c}", [128, S * 128], bf16, kind="ExternalInput")
        for c in range(2)
    ]
    y_d = [
        nc.dram_tensor(f"y{c}", [YGRP, 10, 512], f32, kind="ExternalOutput")
        for c in range(2)
    ]

    with tile.TileContext(nc) as tc:
        with (
            tc.tile_pool(name="const", bufs=1) as const,
            tc.tile_pool(name="hw", bufs=3) as hwp,
            tc.tile_pool(name="ew", bufs=2) as ew,
            tc.tile_pool(name="frp", bufs=2, space="PSUM") as frp,
            tc.tile_pool(name="obp", bufs=2, space="PSUM") as obp,
            tc.tile_pool(name="ubp", bufs=1, space="PSUM") as ubp,
            tc.tile_pool(name="ypsum", bufs=1, space="PSUM") as ypp,
            tc.tile_pool(name="yout", bufs=2) as youtp,
        ):
            wbuf = const.tile([128, KCH * GT * 128], bf16, tag="wbuf")
            nc.sync.dma_start(wbuf[:], w_d[:])
            wxbuf = const.tile([128, G4], bf16, tag="wxbuf")
            nc.sync.dma_start(wxbuf[:], wx_d[:])
            wybuf = const.tile([128, 4 * 10], bf16, tag="wybuf")
            nc.sync.dma_start(wybuf[:], wy_d[:])
            wyb = const.tile([10, 1], f32, tag="wyb")
            nc.sync.dma_start(wyb[:], wyb_d[:])

            xb = []
            cbuf = []
            hprev = []
            for c in range(2):
                t = const.tile([128, S * 128], bf16, tag=f"xb{c}")
                nc.sync.dma_start(t[:], x_d[c][:])
                xb.append(t)
                ct = const.tile([128, H], bf16, tag=f"cbuf{c}")
                nc.vector.memset(ct[:], 0.0)
                cbuf.append(ct)
                ht = hwp.tile([128, 4, 4, 128], bf16, tag=f"hw{c}")
                nc.vector.memset(ht[:], 0.0)
                hprev.append(ht)

            cur = [hprev[0], hprev[1]]
            st = [None, None]  # per-chain (so, hwin tile) between phases

            def mm_block(c, s, prev, bank, gt0, n):
                # per gate tile: 4 h-chunk matmuls, then its x/bias matmul
                # closing the accumulation group (groups must close before
                # the next opens — interleaving corrupts PSUM accumulation)
                jp = (s - 1) % 4
                for i in range(n):
                    gt = gt0 + i
                    out = bank[:, i * 128 : (i + 1) * 128]
                    for k in range(KCH):
                        nc.tensor.matmul(
                            out,
                            lhsT=wbuf[:, (k * GT + gt) * 128 : (k * GT + gt + 1) * 128],
                            rhs=prev[:, k, jp, :],
                            start=(k == 0),
                            stop=False,
                        )
                    rg = gt % PACKX
                    base = PACKW * rg
                    nc.tensor.matmul(
                        out,
                        lhsT=wxbuf[base : base + PACKW, gt * 128 : (gt + 1) * 128],
                        rhs=xb[c][base : base + PACKW, s * 128 : (s + 1) * 128],
                        start=False,
                        stop=True,
                        tile_position=None if PACKX == 1 else (base, 0),
                    )

            def ph1(c, s):
                # u matmuls first (tanh-u unblocks the DVE chain early),
                # then f/r, then o; DVE chain last
                if s % 4 == 0:
                    cur[c] = hwp.tile(
                        [128, 4, 4, 128], bf16, tag=f"hw{c}", name=f"hwc{c}"
                    )
                prev = hprev[c]

                ub = ubp.tile([128, 512], f32, tag="ub")
                mm_block(c, s, prev, ub, 8, 4)
                tu = ew.tile([128, 512], bf16, tag="tu")
                nc.scalar.activation(tu[:], ub[:], AF.Tanh)

                fr = frp.tile([128, 1024], f32, tag="fr")
                mm_block(c, s, prev, fr, 0, 8)
                sfr = ew.tile([128, 1024], bf16, tag="sfr")
                nc.scalar.activation(sfr[:], fr[:], AF.Sigmoid)

                ob = obp.tile([128, 512], f32, tag="ob")
                mm_block(c, s, prev, ob, 12, 4)
                so = ew.tile([128, 512], bf16, tag="so")
                nc.scalar.activation(so[:], ob[:], AF.Sigmoid)

                # late elementwise chain, all on DVE (GpSimd shares its SBUF
                # port with VectorE — concurrent GpSimd work slows DVE ~2.7x):
                #   w = c - tanh(u); fw = f*w; pw = f*fw (= f^2*w)
                #   d = fw - pw (= f(1-f)w); e3 = 2r*d; c' = (tu + pw) + e3
                # equals c' = tu + g*(c - tu) with g = f^2 + 2rf(1-f)
                fg = sfr[:, 0:512]
                rgv = sfr[:, 512:1024]
                w_ = ew.tile([128, 512], bf16, tag="w_")
                fw = ew.tile([128, 512], bf16, tag="fw")
                rg2 = ew.tile([128, 512], bf16, tag="rg2")
                pw = ew.tile([128, 512], bf16, tag="pw")
                dd = ew.tile([128, 512], bf16, tag="dd")
                ca = ew.tile([128, 512], bf16, tag="ca")
                e3 = ew.tile([128, 512], bf16, tag="e3")
                nc.vector.tensor_tensor(w_[:], cbuf[c][:], tu[:], OP.subtract)
                nc.vector.tensor_tensor(fw[:], fg, w_[:], OP.mult)
                nc.vector.tensor_tensor(rg2[:], rgv, rgv, OP.add)
                nc.vector.tensor_tensor(pw[:], fg, fw[:], OP.mult)
                nc.vector.tensor_tensor(dd[:], fw[:], pw[:], OP.subtract)
                nc.vector.tensor_tensor(ca[:], tu[:], pw[:], OP.add)
                nc.vector.tensor_tensor(e3[:], rg2[:], dd[:], OP.mult)
                nc.vector.tensor_tensor(cbuf[c][:], ca[:], e3[:], OP.add)

                st[c] = (so, cur[c])
                hprev[c] = cur[c]

            def ph2h(c, s):
                # tanh(c) and h write for chain c's step s — emitted at the
                # head of the other chain's half so it clears the DVE FIFO
                # before that chain's own late chain queues up
                so, ht = st[c]
                tc2 = ew.tile([128, 512], bf16, tag="tc2")
                nc.scalar.activation(tc2[:], cbuf[c][:], AF.Tanh)
                nc.vector.tensor_tensor(ht[:, :, s % 4, :], so[:], tc2[:], OP.mult)

            def ph2y(c, s):
                # output projection for chain c's step s — PE hits these
                # right between the two chains' matmul phases
                j = s % 4
                g = s // 4
                if not (j == 3 and g >= WGRP):
                    return
                ht = st[c][1]
                yp = ypp.tile([10, 512], f32, tag="yp")
                for k in range(4):
                    nc.tensor.matmul(
                        yp[:],
                        lhsT=wybuf[:, k * 10 : (k + 1) * 10],
                        rhs=ht[:, k, :, :],
                        start=(k == 0),
                        stop=(k == 3),
                    )
                yo = youtp.tile([10, 512], f32, tag="yo")
                nc.vector.tensor_scalar_add(yo[:], yp[:], wyb[:])
                nc.sync.dma_start(y_d[c][g - WGRP], yo[:])

            # interleave: each chain's tanh(c)/h runs at the head of the
            # other chain's matmul phase; y-projection fills the gap
            # between the two phases
            for s in range(S):
                for c in (0, 1):
                    o = 1 - c
                    sprev = s - 1 if c == 0 else s
                    if sprev >= 0:
                        ph2h(o, sprev)
                    ph1(c, s)
                    if sprev >= 0:
                        ph2y(o, sprev)
            ph2h(1, S - 1)
            ph2y(1, S - 1)

    nc.compile()
    return nc


def _prep(inputs):
    x = np.asarray(inputs["x"], np.float32)
    W_ih = np.asarray(inputs["W_ih"], np.float32)
    W_hh = np.asarray(inputs["W_hh"], np.float32)
    b = np.asarray(inputs["b"], np.float32)
    fb = np.asarray(inputs["fb"], np.float32)
    W_out = np.asarray(inputs["W_out"], np.float32)
    b_out = np.asarray(inputs["b_out"], np.float32)
    bf = ml_dtypes.bfloat16

    bias_col = b.copy()
    bias_col[0:H] += fb
    bias_col[H : 2 * H] -= fb

    # h-recurrence weights: [512, 2048] -> per (k, gt) 128x128 lhsT tiles
    w_host = (
        W_hh.T.reshape(KCH, 128, GT, 128).transpose(1, 0, 2, 3).reshape(128, -1)
    ).astype(bf)

    # x/bias weights in row-group strips (strip rg serves gate tiles gt%PACKX==rg)
    wx = np.zeros((128, G4), np.float32)
    for gt in range(GT):
        base = PACKW * (gt % PACKX)
        wx[base : base + I, gt * 128 : (gt + 1) * 128] = W_ih.T[
            :, gt * 128 : (gt + 1) * 128
        ]
        wx[base + I, gt * 128 : (gt + 1) * 128] = bias_col[gt * 128 : (gt + 1) * 128]
    wx_host = wx.astype(bf)

    wy_host = (
        W_out.T.reshape(4, 128, 10).transpose(1, 0, 2).reshape(128, 40).astype(bf)
    )
    wyb_host = b_out.reshape(10, 1).astype(np.float32)

    xc = []
    for jc in range(NCHUNK):
        t0 = jc * C_OUT - W_WARM
        arr = np.zeros((128, S_STEPS * 128), np.float32)
        real0 = max(0, -t0)  # leading pad steps (chunk 0 only)
        xs = x[:, max(t0, 0) : jc * C_OUT + C_OUT, :]  # [128, S-real0, 10]
        a3 = arr.reshape(128, S_STEPS, 128)
        for rg in range(PACKX):
            base = PACKW * rg
            a3[base : base + I, real0:] = xs.transpose(2, 1, 0)
            a3[base + I, real0:] = 1.0
        xc.append(arr.astype(bf))
    return w_host, wx_host, wy_host, wyb_host, xc


def _in_maps(inputs):
    w_host, wx_host, wy_host, wyb_host, xc = _prep(inputs)
    in_maps = []
    for core in range(NCORES):
        in_maps.append(
            {
                "w": w_host,
                "wx": wx_host,
                "wy": wy_host,
                "wyb": wyb_host,
                "x0": xc[2 * core],
                "x1": xc[2 * core + 1],
            }
        )
    return in_maps


def kernel(**inputs):
    from concourse.bass_utils import run_bass_kernel_spmd

    if "nc" not in _cache:
        _cache["nc"] = _build_nc()
    nc = _cache["nc"]

    in_maps = _in_maps(inputs)
    res = run_bass_kernel_spmd(nc, in_maps, list(range(NCORES))).results

    y = np.zeros((B, T, 10), np.float32)
    for jc in range(NCHUNK):
        core, chain = jc // 2, jc % 2
        yj = np.asarray(res[core][f"y{chain}"], np.float32)  # [16, 10, 512]
        yj = yj.reshape(YGRP, 10, 4, 128).transpose(3, 0, 2, 1).reshape(128, C_OUT, 10)
        y[:, jc * C_OUT : (jc + 1) * C_OUT, :] = yj
    return y


# revision 24
# speedup vs baseline: 1.4403x; 1.0476x over previous
"""UR-LSTM forward kernel for Trainium2 (8 NeuronCores).

Strategy (sequence-parallel with warmup):
  The UR-LSTM state is contractive, so a chunk of the sequence can be
  computed to tolerance by starting W steps earlier from zero state.
  T=1024 is split into 16 chunks of 64 output steps; each of the 8 cores
  runs 2 independent chains of S = 64 + 8 steps (chunk 0's 8 warmup
  steps are zero-padded x, which keeps the state exactly zero).

  Per step, per chain (B=128 full batch on every core):
    gates[2048, 128]: per 128-gate tile, 4 h-chunk matmuls plus one K=11
    x/bias matmul that closes the accumulation group (bias b and the
    UR-LSTM fb offsets are folded into a ones row of x).  Accumulation
    groups always close before the next opens — interleaving open groups
    corrupts PSUM.  u gets its own 1-bank PSUM tile (matmuls first, so
    tanh-u lands early), f/r share a 2-bank tile, o a 1-bank tile, all
    in separate pools so the write-after-read on each clears early.
    Elementwise is bf16 throughout (2x DVE mode), with all of the cell
    update on VectorE (GpSimd shares VectorE's SBUF port; using it slows
    concurrent DVE ops ~2.7x).  The chain is factored as
    c' = (tu + f^2 w) + 2r(fw - f^2 w), w = c - tu, so everything
    follows from w/f directly.  The two chains are emitted interleaved:
    each chain's tanh(c)/h runs at the head of the other chain's matmul
    phase, keeping per-engine FIFO order aligned with readiness.
    h_t is written into a [128, k(4), j(4), b(128)] ring tile; every 4
    steps the output projection runs as 4 N=512 matmuls between the two
    chains' phases, with b_out added by a per-partition DVE scalar add.
"""

import numpy as np
import ml_dtypes

B, T, I, H = 128, 1024, 10, 512
G4 = 4 * H  # 2048
NCORES = 8
NCHUNK = 16
C_OUT = T // NCHUNK  # 64
W_WARM = 8
S_STEPS = C_OUT + W_WARM  # 76
NGRP = S_STEPS // 4  # 19
WGRP = W_WARM // 4  # 3 warmup groups
YGRP = NGRP - WGRP  # 16 output groups
KCH = 4  # h-chunks per gate tile (x handled via packed 32-row matmuls)
GT = 16  # gate tiles of 128
PACKX = 1  # concurrent row-group tiles for the x/bias matmuls (1, 2, or 4)
PACKW = 128 // PACKX  # strip width

_cache = {}


def _build_nc():
    import concourse.bacc as bacc
    import concourse.mybir as mybir
    import concourse.tile as tile

    dt = mybir.dt
    f32, bf16 = dt.float32, dt.bfloat16
    AF = mybir.ActivationFunctionType
    OP = mybir.AluOpType
    S = S_STEPS

    nc = bacc.Bacc(None, target_bir_lowering=False)

    w_d = nc.dram_tensor("w", [128, KCH * GT * 128], bf16, kind="ExternalInput")
    wx_d = nc.dram_tensor("wx", [128, G4], bf16, kind="ExternalInput")
    wy_d = nc.dram_tensor("wy", [128, 4 * 10], bf16, kind="ExternalInput")
    wyb_d = nc.dram_tensor("wyb", [10, 1], f32, kind="ExternalInput")
    x_d = [
        nc.dram_tensor(f"x{c}", [128, S * 128], bf16, kind="ExternalInput")
        for c in range(2)
    ]
    y_d = [
        nc.dram_tensor(f"y{c}", [YGRP, 10, 512], f32, kind="ExternalOutput")
        for c in range(2)
    ]

    with tile.TileContext(nc) as tc:
        with (
            tc.tile_pool(name="const", bufs=1) as const,
            tc.tile_pool(name="hw", bufs=3) as hwp,
            tc.tile_pool(name="ew", bufs=2) as ew,
            tc.tile_pool(name="frp", bufs=2, space="PSUM") as frp,
            tc.tile_pool(name="obp", bufs=2, space="PSUM") as obp,
            tc.tile_pool(name="ubp", bufs=1, space="PSUM") as ubp,
            tc.tile_pool(name="ypsum", bufs=1, space="PSUM") as ypp,
            tc.tile_pool(name="yout", bufs=2) as youtp,
        ):
            wbuf = const.tile([128, KCH * GT * 128], bf16, tag="wbuf")
            nc.sync.dma_start(wbuf[:], w_d[:])
            wxbuf = const.tile([128, G4], bf16, tag="wxbuf")
            nc.sync.dma_start(wxbuf[:], wx_d[:])
            wybuf = const.tile([128, 4 * 10], bf16, tag="wybuf")
            nc.sync.dma_start(wybuf[:], wy_d[:])
            wyb = const.tile([10, 1], f32, tag="wyb")
            nc.sync.dma_start(wyb[:], wyb_d[:])

            xb = []
            cbuf = []
            hprev = []
            for c in range(2):
                t = const.tile([128, S * 128], bf16, tag=f"xb{c}")
                nc.sync.dma_start(t[:], x_d[c][:])
                xb.append(t)
                ct = const.tile([128, H], bf16, tag=f"cbuf{c}")
                nc.vector.memset(ct[:], 0.0)
                cbuf.append(ct)
                ht = hwp.tile([128, 4, 4, 128], bf16, tag=f"hw{c}")
                nc.vector.memset(ht[:], 0.0)
                hprev.append(ht)

            cur = [hprev[0], hprev[1]]
            st = [None, None]  # per-chain (so, hwin tile) between phases

            def mm_block(c, s, prev, bank, gt0, n, coff=0):
                # per gate tile: 4 h-chunk matmuls, then its x/bias matmul
                # closing the accumulation group (groups must close before
                # the next opens — interleaving corrupts PSUM accumulation)
                jp = (s - 1) % 4
                for i in range(n):
                    gt = gt0 + i
                    out = bank[:, (coff + i) * 128 : (coff + i + 1) * 128]
                    for k in range(KCH):
                        nc.tensor.matmul(
                            out,
                            lhsT=wbuf[:, (k * GT + gt) * 128 : (k * GT + gt + 1) * 128],
                            rhs=prev[:, k, jp, :],
                            start=(k == 0),
                            stop=False,
                        )
                    rg = gt % PACKX
                    base = PACKW * rg
                    nc.tensor.matmul(
                        out,
                        lhsT=wxbuf[base : base + PACKW, gt * 128 : (gt + 1) * 128],
                        rhs=xb[c][base : base + PACKW, s * 128 : (s + 1) * 128],
                        start=False,
                        stop=True,
                        tile_position=None if PACKX == 1 else (base, 0),
                    )

            def ph1(c, s):
                # u matmuls first (tanh-u unblocks the DVE chain early),
                # then f/r, then o; DVE chain last
                if s % 4 == 0:
                    cur[c] = hwp.tile(
                        [128, 4, 4, 128], bf16, tag=f"hw{c}", name=f"hwc{c}"
                    )
                prev = hprev[c]

                ub = ubp.tile([128, 512], f32, tag="ub")
                mm_block(c, s, prev, ub, 8, 4)
                tu = ew.tile([128, 512], bf16, tag="tu")
                nc.scalar.activation(tu[:], ub[:], AF.Tanh)

                fr = frp.tile([128, 1024], f32, tag="fr")
                sfr = ew.tile([128, 1024], bf16, tag="sfr")
                mm_block(c, s, prev, fr, 0, 4)
                nc.scalar.activation(sfr[:, 0:512], fr[:, 0:512], AF.Sigmoid)
                mm_block(c, s, prev, fr, 4, 4, coff=4)
                nc.scalar.activation(sfr[:, 512:1024], fr[:, 512:1024], AF.Sigmoid)

                ob = obp.tile([128, 512], f32, tag="ob")
                mm_block(c, s, prev, ob, 12, 4)
                so = ew.tile([128, 512], bf16, tag="so")
                nc.scalar.activation(so[:], ob[:], AF.Sigmoid)

                # late elementwise chain, all on DVE (GpSimd shares its SBUF
                # port with VectorE — concurrent GpSimd work slows DVE ~2.7x):
                #   w = c - tanh(u); fw = f*w; pw = f*fw (= f^2*w)
                #   d = fw - pw (= f(1-f)w); e3 = 2r*d; c' = (tu + pw) + e3
                # equals c' = tu + g*(c - tu) with g = f^2 + 2rf(1-f)
                fg = sfr[:, 0:512]
                rgv = sfr[:, 512:1024]
                w_ = ew.tile([128, 512], bf16, tag="w_")
                fw = ew.tile([128, 512], bf16, tag="fw")
                rg2 = ew.tile([128, 512], bf16, tag="rg2")
                pw = ew.tile([128, 512], bf16, tag="pw")
                dd = ew.tile([128, 512], bf16, tag="dd")
                ca = ew.tile([128, 512], bf16, tag="ca")
                e3 = ew.tile([128, 512], bf16, tag="e3")
                nc.vector.tensor_tensor(w_[:], cbuf[c][:], tu[:], OP.subtract)
                nc.vector.tensor_tensor(fw[:], fg, w_[:], OP.mult)
                nc.vector.tensor_tensor(rg2[:], rgv, rgv, OP.add)
                nc.vector.tensor_tensor(pw[:], fg, fw[:], OP.mult)
                nc.vector.tensor_tensor(dd[:], fw[:], pw[:], OP.subtract)
                nc.vector.tensor_tensor(ca[:], tu[:], pw[:], OP.add)
                nc.vector.tensor_tensor(e3[:], rg2[:], dd[:], OP.mult)
                nc.vector.tensor_tensor(cbuf[c][:], ca[:], e3[:], OP.add)

                st[c] = (so, cur[c])
                hprev[c] = cur[c]

            def ph2h(c, s):
                # tanh(c) and h write for chain c's step s — emitted at the
                # head of the other chain's half so it clears the DVE FIFO
                # before that chain's own late chain queues up
                so, ht = st[c]
                tc2 = ew.tile([128, 512], bf16, tag="tc2")
                nc.scalar.activation(tc2[:], cbuf[c][:], AF.Tanh)
                nc.vector.tensor_tensor(ht[:, :, s % 4, :], so[:], tc2[:], OP.mult)

            def ph2y(c, s):
                # output projection for chain c's step s — PE hits these
                # right between the two chains' matmul phases
                j = s % 4
                g = s // 4
                if not (j == 3 and g >= WGRP):
                    return
                ht = st[c][1]
                yp = ypp.tile([10, 512], f32, tag="yp")
                for k in range(4):
                    nc.tensor.matmul(
                        yp[:],
                        lhsT=wybuf[:, k * 10 : (k + 1) * 10],
                        rhs=ht[:, k, :, :],
                        start=(k == 0),
                        stop=(k == 3),
                    )
                yo = youtp.tile([10, 512], f32, tag="yo")
                nc.vector.tensor_scalar_add(yo[:], yp[:], wyb[:])
                nc.sync.dma_start(y_d[c][g - WGRP], yo[:])

            # interleave: each chain's tanh(c)/h runs at the head of the
            # other chain's matmul phase; y-projection fills the gap
            # between the two phases
            for s in range(S):
                for c in (0, 1):
                    o = 1 - c
                    sprev = s - 1 if c == 0 else s
                    if sprev >= 0:
                        ph2h(o, sprev)
                    ph1(c, s)
                    if sprev >= 0:
                        ph2y(o, sprev)
            ph2h(1, S - 1)
            ph2y(1, S - 1)

    nc.compile()
    return nc


def _prep(inputs):
    x = np.asarray(inputs["x"], np.float32)
    W_ih = np.asarray(inputs["W_ih"], np.float32)
    W_hh = np.asarray(inputs["W_hh"], np.float32)
    b = np.asarray(inputs["b"], np.float32)
    fb = np.asarray(inputs["fb"], np.float32)
    W_out = np.asarray(inputs["W_out"], np.float32)
    b_out = np.asarray(inputs["b_out"], np.float32)
    bf = ml_dtypes.bfloat16

    bias_col = b.copy()
    bias_col[0:H] += fb
    bias_col[H : 2 * H] -= fb

    # h-recurrence weights: [512, 2048] -> per (k, gt) 128x128 lhsT tiles
    w_host = (
        W_hh.T.reshape(KCH, 128, GT, 128).transpose(1, 0, 2, 3).reshape(128, -1)
    ).astype(bf)

    # x/bias weights in row-group strips (strip rg serves gate tiles gt%PACKX==rg)
    wx = np.zeros((128, G4), np.float32)
    for gt in range(GT):
        base = PACKW * (gt % PACKX)
        wx[base : base + I, gt * 128 : (gt + 1) * 128] = W_ih.T[
            :, gt * 128 : (gt + 1) * 128
        ]
        wx[base + I, gt * 128 : (gt + 1) * 128] = bias_col[gt * 128 : (gt + 1) * 128]
    wx_host = wx.astype(bf)

    wy_host = (
        W_out.T.reshape(4, 128, 10).transpose(1, 0, 2).reshape(128, 40).astype(bf)
    )
    wyb_host = b_out.reshape(10, 1).astype(np.float32)

    xc = []
    for jc in range(NCHUNK):
        t0 = jc * C_OUT - W_WARM
        arr = np.zeros((128, S_STEPS * 128), np.float32)
        real0 = max(0, -t0)  # leading pad steps (chunk 0 only)
        xs = x[:, max(t0, 0) : jc * C_OUT + C_OUT, :]  # [128, S-real0, 10]
        a3 = arr.reshape(128, S_STEPS, 128)
        for rg in range(PACKX):
            base = PACKW * rg
            a3[base : base + I, real0:] = xs.transpose(2, 1, 0)
            a3[base + I, real0:] = 1.0
        xc.append(arr.astype(bf))
    return w_host, wx_host, wy_host, wyb_host, xc


def _in_maps(inputs):
    w_host, wx_host, wy_host, wyb_host, xc = _prep(inputs)
    in_maps = []
    for core in range(NCORES):
        in_maps.append(
            {
                "w": w_host,
                "wx": wx_host,
                "wy": wy_host,
                "wyb": wyb_host,
                "x0": xc[2 * core],
                "x1": xc[2 * core + 1],
            }
        )
    return in_maps


def kernel(**inputs):
    from concourse.bass_utils import run_bass_kernel_spmd

    if "nc" not in _cache:
        _cache["nc"] = _build_nc()
    nc = _cache["nc"]

    in_maps = _in_maps(inputs)
    res = run_bass_kernel_spmd(nc, in_maps, list(range(NCORES))).results

    y = np.zeros((B, T, 10), np.float32)
    for jc in range(NCHUNK):
        core, chain = jc // 2, jc % 2
        yj = np.asarray(res[core][f"y{chain}"], np.float32)  # [16, 10, 512]
        yj = yj.reshape(YGRP, 10, 4, 128).transpose(3, 0, 2, 1).reshape(128, C_OUT, 10)
        y[:, jc * C_OUT : (jc + 1) * C_OUT, :] = yj
    return y
